# revision 15
# baseline (speedup 1.0000x reference)
"""CNF forward (vector field + exact Jacobian trace) on 8 TRN2 cores.

Math: reference computes, per sample x (row of state[:, 1:]):
    f(x)  = W3^T tanh(W2^T tanh(W1^T [x; t] + b1) + b2) + b3      (dx)
    trJ   = trace(df/dx)                                          (aug = -trJ)

Closed form of the trace (no JVP loop):
    h1 = tanh([x;t] @ W1 + b1),  h2 = tanh(h1 @ W2 + b2)
    s1 = 1 - h1^2,               s2 = 1 - h2^2
    trJ = s1^T F s2   with  F[h',h] = W2[h',h] * (W3 @ W1[:D])[h, h']

Sharding: data-parallel, 128 samples per core, weights replicated.

All matmuls run in bf16 (1 PE cycle/row vs fp32's 4), accumulating in
fp32 PSUM; host-side work is layout/dtype-only. The b1/t bias rides the
contraction as a K=65 ones-row, so layer 1 is four back-to-back matmuls
into one PSUM bank finished by a single 512-wide tanh. Weights stream as
five DMA blocks over three engine rings: the layer-1 block first, W2 in
four 128KB tiles whose completion fences fire as each tile lands (a tiny
identity shim keeps their descriptors from head-of-line blocking the
layer-1 block). PE order: z1 -> e2t (W2 still in flight) -> z2 -> t2 ->
h2 transposes -> dx; t2_k chases F_k so the trace matmul starts before
the last F tile exists. Engine split honoring "gpsimd can't read PSUM":
DVE does F = W2 .* E2T and the trace product+reduce, gpsimd does the
tanh' terms from SBUF, ACT does tanhs and all PSUM->SBUF copies.

NOTE: DVE tensor_tensor_reduce faults real HW (NRT unrecoverable;
CoreSim accepts it) — use the tensor_mul + tensor_reduce pair instead.
"""

import numpy as np
import ml_dtypes

import concourse.bacc as bacc
import concourse.bass as bass
import concourse.tile as tile
from concourse import mybir
from concourse.bass_utils import run_bass_kernel_spmd
from concourse.tile_rust import add_dep_helper

B, D, H = 1024, 64, 512
NCORES = 8
BC = B // NCORES  # 128 samples per core
KT = H // 128     # 4 feature tiles of 128
F32 = mybir.dt.float32
BF16 = mybir.dt.bfloat16
AF = mybir.ActivationFunctionType
ALU = mybir.AluOpType
ts = bass.ts
BF = ml_dtypes.bfloat16

_NC = {}

# blkA column offsets: xT | w1ext  (65 partitions, bf16).
# Row 64 is the bias row: 1s under xT, b1+t*W1[D] under w1ext.
XT_OFF = 0
W1X_OFF = BC            # 128
BLKA_COLS = BC + H      # 640


def _build(with_bias23: bool):
    """with_bias23: include rank-1 bias adds for b2/b3 (batch-major layers
    can't take a per-free-dim bias via ACT). setup_inputs() has zero
    biases so the fast path skips them; nonzero b2/b3 still works."""
    nc = bacc.Bacc()

    blkA = nc.declare_dram_parameter("blkA", [D + 1, BLKA_COLS], BF16,
                                     isOutput=False)
    w3Tp = nc.declare_dram_parameter("w3Tp", [D, H], BF16, isOutput=False)
    w2d = [nc.declare_dram_parameter(f"w2_{k}", [128, H], BF16,
                                     isOutput=False) for k in range(3)]
    # w2_3 | w3 tiles side by side
    w2e = nc.declare_dram_parameter("w2e", [128, H + KT * D], BF16,
                                    isOutput=False)
    identB = nc.declare_dram_parameter("identB", [128, 128], BF16,
                                       isOutput=False)
    if with_bias23:
        b2r = nc.declare_dram_parameter("b2r", [1, H], BF16, isOutput=False)
        b3r = nc.declare_dram_parameter("b3r", [1, D], BF16, isOutput=False)
    out = nc.declare_dram_parameter("out", [BC, D + 1], F32, isOutput=True)

    with tile.TileContext(nc) as tc:
        with (
            tc.tile_pool(name="const", bufs=1) as cp,
            tc.tile_pool(name="act", bufs=1) as ap,
            tc.tile_pool(name="ps", bufs=1, space="PSUM") as ps,
        ):
            # ---- loads ----
            # sync: blkA (fast DIRECT2D gen), w2_2, out store
            # scalar: w3T (needed by e2t), W2 halves, ident
            # gpsimd: w2_3|w3 via SWDGE (generation overlaps the blkA wait)
            blkA_sb = cp.tile([D + 1, BLKA_COLS], BF16, tag="blkA")
            w3T_sb = cp.tile([D, H], BF16, tag="w3T")
            ident = cp.tile([128, 128], BF16, tag="ident")
            w2_sb = [cp.tile([128, H], BF16, tag=f"w2s_{k}", name=f"w2s_{k}")
                     for k in range(3)]
            w2e_sb = cp.tile([128, H + KT * D], BF16, tag="w2e")
            nc.scalar.dma_start(out=blkA_sb, in_=blkA[:, :])
            nc.sync.dma_start(out=w3T_sb, in_=w3Tp[:, :])
            nc.scalar.dma_start(out=w2_sb[0], in_=w2d[0][:, :])
            nc.scalar.dma_start(out=w2_sb[1], in_=w2d[1][:, :])
            nc.sync.dma_start(out=w2_sb[2], in_=w2d[2][:, :])
            nc.gpsimd.dma_start(out=w2e_sb, in_=w2e[:, :])
            nc.scalar.dma_start(out=ident, in_=identB[:, :])
            w2_sb.append(w2e_sb[:, 0:H])
            w3_sb = [w2e_sb[:, H + k * D:H + (k + 1) * D] for k in range(KT)]
            if with_bias23:
                b2r_sb = cp.tile([1, H], BF16, tag="b2r")
                nc.sync.dma_start(out=b2r_sb, in_=b2r[:, :])
                b3r_sb = cp.tile([1, D], BF16, tag="b3r")
                nc.sync.dma_start(out=b3r_sb, in_=b3r[:, :])
                onesr = cp.tile([1, BC], BF16, tag="onesr")
                nc.vector.memset(onesr, 1.0)
            xTe = blkA_sb[:, XT_OFF:XT_OFF + BC]            # [65, BC]
            w1e = blkA_sb[:, W1X_OFF:W1X_OFF + H]           # [65, H]
            w1x = blkA_sb[0:D, W1X_OFF:W1X_OFF + H]         # [64, H]
            w3T = w3T_sb[:, :]                              # [64, H]

            # ---- layer 1 (feature-major): 4 matmuls, one tanh ----
            z1_ps = ps.tile([128, KT * BC], F32, tag="z1", bufs=1)
            z1_mm = []
            for j in range(KT):
                z1_mm.append(
                    nc.tensor.matmul(z1_ps[:, ts(j, BC)], w1e[:, ts(j, 128)],
                                     xTe, start=True, stop=True))
            h1 = ap.tile([128, KT * BC], BF16, tag="h1")
            s1 = ap.tile([128, KT * BC], BF16, tag="s1")
            nc.scalar.activation(h1, z1_ps, AF.Tanh)
            nc.gpsimd.tensor_mul(s1, h1, h1)
            nc.gpsimd.tensor_scalar(s1, s1, -1.0, 1.0, ALU.mult, ALU.add)

            # ---- trace weight matrix F (runs while W2 is in flight) ----
            # ACT drains each e2t PSUM slot to bf16 SBUF (faster than a DVE
            # read-modify chain would free it), then the DVE multiply runs
            # all-bf16 (2x mode) off the critical path.
            f_sb = []
            e2t_mm = []
            for m in range(KT):
                e2t_ps = ps.tile([128, H], F32, tag="e2t", bufs=2)
                mm = nc.tensor.matmul(e2t_ps, w1x[:, ts(m, 128)], w3T,
                                      start=True, stop=True)
                if m == 0:
                    add_dep_helper(mm.ins, z1_mm[KT - 1].ins, sync=False,
                                   reason="pe-order e2t after z1")
                e2t_mm.append(mm)
                e2c = ap.tile([128, H], BF16, tag=f"e2c_{m}", name=f"e2c_{m}")
                if m % 2 == 0:
                    nc.vector.tensor_copy(e2c, e2t_ps)
                else:
                    nc.scalar.copy(e2c, e2t_ps)
                fm = ap.tile([128, H], BF16, tag=f"f_{m}")
                nc.vector.tensor_mul(fm, w2_sb[m], e2c)
                f_sb.append(fm)

            # ---- layer 2 (batch-major): h2, s2 ----
            z2_ps = ps.tile([BC, H], F32, tag="z2", bufs=1)
            z2_mm = []
            for k in range(KT):
                z2_mm.append(
                    nc.tensor.matmul(z2_ps, h1[:, ts(k, BC)], w2_sb[k],
                                     start=(k == 0),
                                     stop=(k == KT - 1 and not with_bias23)))
            add_dep_helper(z2_mm[0].ins, e2t_mm[KT - 1].ins, sync=False,
                           reason="pe-order z2 after e2t")
            if with_bias23:
                nc.tensor.matmul(z2_ps, onesr, b2r_sb, start=False, stop=True)
            h2 = ap.tile([BC, H], BF16, tag="h2")
            s2 = ap.tile([BC, H], BF16, tag="s2")
            HH = H // 2
            # tanh in halves so the first transpose starts sooner
            nc.scalar.activation(h2[:, 0:HH], z2_ps[:, 0:HH], AF.Tanh)
            nc.scalar.activation(h2[:, HH:H], z2_ps[:, HH:H], AF.Tanh)
            # s2 on DVE (all-bf16, 2x-eligible), chained after each tanh
            # half so the trace product is never waiting on it
            nc.vector.tensor_mul(s2[:, 0:HH], h2[:, 0:HH], h2[:, 0:HH])
            nc.vector.tensor_scalar(s2[:, 0:HH], s2[:, 0:HH],
                                    -1.0, 1.0, ALU.mult, ALU.add)
            nc.vector.tensor_mul(s2[:, HH:H], h2[:, HH:H], h2[:, HH:H])
            nc.vector.tensor_scalar(s2[:, HH:H], s2[:, HH:H],
                                    -1.0, 1.0, ALU.mult, ALU.add)

            # ---- trJ = s1^T F s2 (batch-major) ----
            t2_ps = ps.tile([BC, H], F32, tag="t2", bufs=1)
            t2_mm = []
            for k in range(KT):
                t2_mm.append(
                    nc.tensor.matmul(t2_ps, s1[:, ts(k, BC)], f_sb[k],
                                     start=(k == 0), stop=(k == KT - 1)))
            add_dep_helper(t2_mm[0].ins, z2_mm[KT - 1].ins, sync=False,
                           reason="pe-order t2 after z2")
            final_sb = ap.tile([BC, D + 1], F32, tag="final")
            ttr_scr = ap.tile([BC, H], BF16, tag="ttr_scr")
            nc.vector.tensor_mul(ttr_scr, t2_ps, s2)
            nc.vector.tensor_reduce(out=final_sb[:, 0:1], in_=ttr_scr,
                                    op=ALU.add, axis=mybir.AxisListType.X,
                                    negate=True)

            # ---- layer 3 (batch-major): dx ----
            # double-buffered transpose PSUM; copies on ACT (the DVE owns
            # the trace product/reduce in this window)
            h2T_sb = []
            tr_mm = []
            for j in range(KT):
                hT_ps = ps.tile([128, BC], BF16, tag="tr", bufs=2)
                mm = nc.tensor.transpose(hT_ps, h2[:, ts(j, 128)], ident)
                if j == 0:
                    add_dep_helper(mm.ins, t2_mm[KT - 1].ins, sync=False,
                                   reason="pe-order transposes after t2")
                tr_mm.append(mm)
                hT = ap.tile([128, BC], BF16, tag=f"h2T_{j}", name=f"hT_{j}")
                nc.scalar.copy(hT, hT_ps)
                h2T_sb.append(hT)
            o_ps = ps.tile([BC, D], F32, tag="o", bufs=1)
            for k in range(KT):
                nc.tensor.matmul(o_ps, h2T_sb[k], w3_sb[k],
                                 start=(k == 0),
                                 stop=(k == KT - 1 and not with_bias23))
            if with_bias23:
                nc.tensor.matmul(o_ps, onesr, b3r_sb, start=False, stop=True)
            nc.scalar.copy(final_sb[:, 1:D + 1], o_ps)
            nc.sync.dma_start(out=out[:, :], in_=final_sb)

    nc.finalize()
    return nc


def _get_nc(with_bias23: bool):
    key = bool(with_bias23)
    if key not in _NC:
        _NC[key] = _build(key)
    return _NC[key]


def make_in_maps(inputs):
    f32 = lambda a: np.ascontiguousarray(np.asarray(a), dtype=np.float32)
    bfc = lambda a: np.ascontiguousarray(a.astype(BF))
    state = f32(inputs["state"])
    t = float(np.asarray(inputs["t"]).reshape(-1)[0])
    W1 = f32(inputs["W1"])
    b1 = f32(inputs["b1"]).reshape(H)
    W2 = f32(inputs["W2"])
    b2 = f32(inputs["b2"]).reshape(H)
    W3 = f32(inputs["W3"])
    b3 = f32(inputs["b3"]).reshape(D)

    with_bias23 = bool(np.any(b2) or np.any(b3))

    b1_eff = b1 + t * W1[D]                       # fold t-row into bias
    w1e = np.concatenate([W1[:D], b1_eff[None, :]], axis=0)   # [65, H]
    w2b = W2.astype(BF)
    w3blk = np.concatenate([W3[k * 128:(k + 1) * 128] for k in range(KT)],
                           axis=1).astype(BF)     # [128, KT*D]
    base = {
        "w2_0": np.ascontiguousarray(w2b[0:128]),
        "w2_1": np.ascontiguousarray(w2b[128:256]),
        "w2_2": np.ascontiguousarray(w2b[256:384]),
        "w2e": np.ascontiguousarray(
            np.concatenate([w2b[384:512], w3blk], axis=1)),
        "identB": np.eye(128, dtype=BF),
        "w3Tp": np.ascontiguousarray(W3.T).astype(BF),
    }
    if with_bias23:
        base["b2r"] = bfc(b2.reshape(1, H))
        base["b3r"] = bfc(b3.reshape(1, D))
    ones_row = np.ones((1, BC), np.float32)
    in_maps = []
    for c in range(NCORES):
        m = dict(base)
        xT = state[c * BC:(c + 1) * BC, 1:].T                 # [64, BC]
        xTe = np.concatenate([xT, ones_row], axis=0)          # [65, BC]
        m["blkA"] = np.ascontiguousarray(np.concatenate(
            [xTe, w1e], axis=1).astype(BF))
        in_maps.append(m)
    return with_bias23, in_maps


def kernel(**inputs) -> np.ndarray:
    with_bias23, in_maps = make_in_maps(inputs)
    res = run_bass_kernel_spmd(_get_nc(with_bias23), in_maps,
                               list(range(NCORES))).results
    return np.concatenate([res[c]["out"] for c in range(NCORES)], axis=0)


# revision 16
# speedup vs baseline: 1.0186x; 1.0186x over previous
"""CNF forward (vector field + exact Jacobian trace) on 8 TRN2 cores.

Math: reference computes, per sample x (row of state[:, 1:]):
    f(x)  = W3^T tanh(W2^T tanh(W1^T [x; t] + b1) + b2) + b3      (dx)
    trJ   = trace(df/dx)                                          (aug = -trJ)

Closed form of the trace (no JVP loop):
    h1 = tanh([x;t] @ W1 + b1),  h2 = tanh(h1 @ W2 + b2)
    s1 = 1 - h1^2,               s2 = 1 - h2^2
    trJ = s1^T F s2   with  F[h',h] = W2[h',h] * (W3 @ W1[:D])[h, h']

Sharding: data-parallel, 128 samples per core, weights replicated.

All matmuls run in bf16 (1 PE cycle/row vs fp32's 4), accumulating in
fp32 PSUM; host-side work is layout/dtype-only. The b1/t bias rides the
contraction as a K=65 ones-row, so layer 1 is four back-to-back matmuls
into one PSUM bank finished by a single 512-wide tanh. Weights stream as
five DMA blocks over three engine rings: the layer-1 block first, W2 in
four 128KB tiles whose completion fences fire as each tile lands (a tiny
identity shim keeps their descriptors from head-of-line blocking the
layer-1 block). PE order: z1 -> e2t (W2 still in flight) -> z2 -> t2 ->
h2 transposes -> dx; t2_k chases F_k so the trace matmul starts before
the last F tile exists. Engine split honoring "gpsimd can't read PSUM":
DVE does F = W2 .* E2T and the trace product+reduce, gpsimd does the
tanh' terms from SBUF, ACT does tanhs and all PSUM->SBUF copies.

NOTE: DVE tensor_tensor_reduce faults real HW (NRT unrecoverable;
CoreSim accepts it) — use the tensor_mul + tensor_reduce pair instead.
"""

import numpy as np
import ml_dtypes

import concourse.bacc as bacc
import concourse.bass as bass
import concourse.tile as tile
from concourse import mybir
from concourse.bass_utils import run_bass_kernel_spmd
from concourse.tile_rust import add_dep_helper

B, D, H = 1024, 64, 512
NCORES = 8
BC = B // NCORES  # 128 samples per core
KT = H // 128     # 4 feature tiles of 128
F32 = mybir.dt.float32
BF16 = mybir.dt.bfloat16
AF = mybir.ActivationFunctionType
ALU = mybir.AluOpType
ts = bass.ts
BF = ml_dtypes.bfloat16

_NC = {}

# blkA column offsets: xT | w1ext  (65 partitions, bf16).
# Row 64 is the bias row: 1s under xT, b1+t*W1[D] under w1ext.
XT_OFF = 0
W1X_OFF = BC            # 128
BLKA_COLS = BC + H      # 640


def _build(with_bias23: bool):
    """with_bias23: include rank-1 bias adds for b2/b3 (batch-major layers
    can't take a per-free-dim bias via ACT). setup_inputs() has zero
    biases so the fast path skips them; nonzero b2/b3 still works."""
    nc = bacc.Bacc()

    blkA = nc.declare_dram_parameter("blkA", [D + 1, BLKA_COLS], BF16,
                                     isOutput=False)
    w3Tp = nc.declare_dram_parameter("w3Tp", [D, H], BF16, isOutput=False)
    w2d = [nc.declare_dram_parameter(f"w2_{k}", [128, H], BF16,
                                     isOutput=False) for k in range(3)]
    # w2_3 | w3 tiles side by side
    w2e = nc.declare_dram_parameter("w2e", [128, H + KT * D], BF16,
                                    isOutput=False)
    identB = nc.declare_dram_parameter("identB", [128, 128], BF16,
                                       isOutput=False)
    if with_bias23:
        b2r = nc.declare_dram_parameter("b2r", [1, H], BF16, isOutput=False)
        b3r = nc.declare_dram_parameter("b3r", [1, D], BF16, isOutput=False)
    out = nc.declare_dram_parameter("out", [BC, D + 1], F32, isOutput=True)

    with tile.TileContext(nc) as tc:
        with (
            tc.tile_pool(name="const", bufs=1) as cp,
            tc.tile_pool(name="act", bufs=1) as ap,
            tc.tile_pool(name="ps", bufs=1, space="PSUM") as ps,
        ):
            # ---- loads ----
            # sync: blkA (fast DIRECT2D gen), w2_2, out store
            # scalar: w3T (needed by e2t), W2 halves, ident
            # gpsimd: w2_3|w3 via SWDGE (generation overlaps the blkA wait)
            blkA_sb = cp.tile([D + 1, BLKA_COLS], BF16, tag="blkA")
            w3T_sb = cp.tile([D, H], BF16, tag="w3T")
            ident = cp.tile([128, 128], BF16, tag="ident")
            w2_sb = [cp.tile([128, H], BF16, tag=f"w2s_{k}", name=f"w2s_{k}")
                     for k in range(3)]
            w2e_sb = cp.tile([128, H + KT * D], BF16, tag="w2e")
            nc.sync.dma_start(out=blkA_sb, in_=blkA[:, :])
            nc.scalar.dma_start(out=w3T_sb, in_=w3Tp[:, :])
            nc.scalar.dma_start(out=w2_sb[0], in_=w2d[0][:, :])
            nc.scalar.dma_start(out=w2_sb[1], in_=w2d[1][:, :])
            nc.sync.dma_start(out=w2_sb[2], in_=w2d[2][:, :])
            nc.gpsimd.dma_start(out=w2e_sb, in_=w2e[:, :])
            nc.scalar.dma_start(out=ident, in_=identB[:, :])
            w2_sb.append(w2e_sb[:, 0:H])
            w3_sb = [w2e_sb[:, H + k * D:H + (k + 1) * D] for k in range(KT)]
            if with_bias23:
                b2r_sb = cp.tile([1, H], BF16, tag="b2r")
                nc.sync.dma_start(out=b2r_sb, in_=b2r[:, :])
                b3r_sb = cp.tile([1, D], BF16, tag="b3r")
                nc.sync.dma_start(out=b3r_sb, in_=b3r[:, :])
                onesr = cp.tile([1, BC], BF16, tag="onesr")
                nc.vector.memset(onesr, 1.0)
            xTe = blkA_sb[:, XT_OFF:XT_OFF + BC]            # [65, BC]
            w1e = blkA_sb[:, W1X_OFF:W1X_OFF + H]           # [65, H]
            w1x = blkA_sb[0:D, W1X_OFF:W1X_OFF + H]         # [64, H]
            w3T = w3T_sb[:, :]                              # [64, H]

            # ---- layer 1 (feature-major): 4 matmuls, one tanh ----
            z1_ps = ps.tile([128, KT * BC], F32, tag="z1", bufs=1)
            z1_mm = []
            for j in range(KT):
                z1_mm.append(
                    nc.tensor.matmul(z1_ps[:, ts(j, BC)], w1e[:, ts(j, 128)],
                                     xTe, start=True, stop=True))
            h1 = ap.tile([128, KT * BC], BF16, tag="h1")
            s1 = ap.tile([128, KT * BC], BF16, tag="s1")
            nc.scalar.activation(h1, z1_ps, AF.Tanh)
            nc.gpsimd.tensor_mul(s1, h1, h1)
            nc.gpsimd.tensor_scalar(s1, s1, -1.0, 1.0, ALU.mult, ALU.add)

            # ---- trace weight matrix F (runs while W2 is in flight) ----
            # ACT drains each e2t PSUM slot to bf16 SBUF (faster than a DVE
            # read-modify chain would free it), then the DVE multiply runs
            # all-bf16 (2x mode) off the critical path.
            f_sb = []
            e2t_mm = []
            for m in range(KT):
                e2t_ps = ps.tile([128, H], F32, tag="e2t", bufs=2)
                mm = nc.tensor.matmul(e2t_ps, w1x[:, ts(m, 128)], w3T,
                                      start=True, stop=True)
                if m == 0:
                    add_dep_helper(mm.ins, z1_mm[KT - 1].ins, sync=False,
                                   reason="pe-order e2t after z1")
                e2t_mm.append(mm)
                e2c = ap.tile([128, H], BF16, tag=f"e2c_{m}", name=f"e2c_{m}")
                if m % 2 == 0:
                    nc.vector.tensor_copy(e2c, e2t_ps)
                else:
                    nc.scalar.copy(e2c, e2t_ps)
                fm = ap.tile([128, H], BF16, tag=f"f_{m}")
                nc.vector.tensor_mul(fm, w2_sb[m], e2c)
                f_sb.append(fm)

            # ---- layer 2 (batch-major): h2, s2 ----
            z2_ps = ps.tile([BC, H], F32, tag="z2", bufs=1)
            z2_mm = []
            for k in range(KT):
                z2_mm.append(
                    nc.tensor.matmul(z2_ps, h1[:, ts(k, BC)], w2_sb[k],
                                     start=(k == 0),
                                     stop=(k == KT - 1 and not with_bias23)))
            add_dep_helper(z2_mm[0].ins, e2t_mm[KT - 1].ins, sync=False,
                           reason="pe-order z2 after e2t")
            if with_bias23:
                nc.tensor.matmul(z2_ps, onesr, b2r_sb, start=False, stop=True)
            h2 = ap.tile([BC, H], BF16, tag="h2")
            s2 = ap.tile([BC, H], BF16, tag="s2")
            HH = H // 2
            # tanh in halves so the first transpose starts sooner
            nc.scalar.activation(h2[:, 0:HH], z2_ps[:, 0:HH], AF.Tanh)
            nc.scalar.activation(h2[:, HH:H], z2_ps[:, HH:H], AF.Tanh)
            # s2 on DVE (all-bf16, 2x-eligible), chained after each tanh
            # half so the trace product is never waiting on it
            nc.vector.tensor_mul(s2[:, 0:HH], h2[:, 0:HH], h2[:, 0:HH])
            nc.vector.tensor_scalar(s2[:, 0:HH], s2[:, 0:HH],
                                    -1.0, 1.0, ALU.mult, ALU.add)
            nc.vector.tensor_mul(s2[:, HH:H], h2[:, HH:H], h2[:, HH:H])
            nc.vector.tensor_scalar(s2[:, HH:H], s2[:, HH:H],
                                    -1.0, 1.0, ALU.mult, ALU.add)

            # ---- trJ = s1^T F s2 (batch-major) ----
            t2_ps = ps.tile([BC, H], F32, tag="t2", bufs=1)
            t2_mm = []
            for k in range(KT):
                t2_mm.append(
                    nc.tensor.matmul(t2_ps, s1[:, ts(k, BC)], f_sb[k],
                                     start=(k == 0), stop=(k == KT - 1)))
            add_dep_helper(t2_mm[0].ins, z2_mm[KT - 1].ins, sync=False,
                           reason="pe-order t2 after z2")
            final_sb = ap.tile([BC, D + 1], F32, tag="final")
            ttr_scr = ap.tile([BC, H], BF16, tag="ttr_scr")
            nc.vector.tensor_mul(ttr_scr, t2_ps, s2)
            nc.vector.tensor_reduce(out=final_sb[:, 0:1], in_=ttr_scr,
                                    op=ALU.add, axis=mybir.AxisListType.X,
                                    negate=True)

            # ---- layer 3 (batch-major): dx ----
            # double-buffered transpose PSUM; copies on ACT (the DVE owns
            # the trace product/reduce in this window)
            h2T_sb = []
            tr_mm = []
            for j in range(KT):
                hT_ps = ps.tile([128, BC], BF16, tag="tr", bufs=2)
                mm = nc.tensor.transpose(hT_ps, h2[:, ts(j, 128)], ident)
                if j == 0:
                    add_dep_helper(mm.ins, t2_mm[KT - 1].ins, sync=False,
                                   reason="pe-order transposes after t2")
                tr_mm.append(mm)
                hT = ap.tile([128, BC], BF16, tag=f"h2T_{j}", name=f"hT_{j}")
                nc.scalar.copy(hT, hT_ps)
                h2T_sb.append(hT)
            o_ps = ps.tile([BC, D], F32, tag="o", bufs=1)
            for k in range(KT):
                nc.tensor.matmul(o_ps, h2T_sb[k], w3_sb[k],
                                 start=(k == 0),
                                 stop=(k == KT - 1 and not with_bias23))
            if with_bias23:
                nc.tensor.matmul(o_ps, onesr, b3r_sb, start=False, stop=True)
            nc.scalar.copy(final_sb[:, 1:D + 1], o_ps)
            nc.sync.dma_start(out=out[:, :], in_=final_sb)

    nc.finalize()
    return nc


def _get_nc(with_bias23: bool):
    key = bool(with_bias23)
    if key not in _NC:
        _NC[key] = _build(key)
    return _NC[key]


def make_in_maps(inputs):
    f32 = lambda a: np.ascontiguousarray(np.asarray(a), dtype=np.float32)
    bfc = lambda a: np.ascontiguousarray(a.astype(BF))
    state = f32(inputs["state"])
    t = float(np.asarray(inputs["t"]).reshape(-1)[0])
    W1 = f32(inputs["W1"])
    b1 = f32(inputs["b1"]).reshape(H)
    W2 = f32(inputs["W2"])
    b2 = f32(inputs["b2"]).reshape(H)
    W3 = f32(inputs["W3"])
    b3 = f32(inputs["b3"]).reshape(D)

    with_bias23 = bool(np.any(b2) or np.any(b3))

    b1_eff = b1 + t * W1[D]                       # fold t-row into bias
    w1e = np.concatenate([W1[:D], b1_eff[None, :]], axis=0)   # [65, H]
    w2b = W2.astype(BF)
    w3blk = np.concatenate([W3[k * 128:(k + 1) * 128] for k in range(KT)],
                           axis=1).astype(BF)     # [128, KT*D]
    base = {
        "w2_0": np.ascontiguousarray(w2b[0:128]),
        "w2_1": np.ascontiguousarray(w2b[128:256]),
        "w2_2": np.ascontiguousarray(w2b[256:384]),
        "w2e": np.ascontiguousarray(
            np.concatenate([w2b[384:512], w3blk], axis=1)),
        "identB": np.eye(128, dtype=BF),
        "w3Tp": np.ascontiguousarray(W3.T).astype(BF),
    }
    if with_bias23:
        base["b2r"] = bfc(b2.reshape(1, H))
        base["b3r"] = bfc(b3.reshape(1, D))
    ones_row = np.ones((1, BC), np.float32)
    in_maps = []
    for c in range(NCORES):
        m = dict(base)
        xT = state[c * BC:(c + 1) * BC, 1:].T                 # [64, BC]
        xTe = np.concatenate([xT, ones_row], axis=0)          # [65, BC]
        m["blkA"] = np.ascontiguousarray(np.concatenate(
            [xTe, w1e], axis=1).astype(BF))
        in_maps.append(m)
    return with_bias23, in_maps


def kernel(**inputs) -> np.ndarray:
    with_bias23, in_maps = make_in_maps(inputs)
    res = run_bass_kernel_spmd(_get_nc(with_bias23), in_maps,
                               list(range(NCORES))).results
    return np.concatenate([res[c]["out"] for c in range(NCORES)], axis=0)


# revision 17
# speedup vs baseline: 1.0411x; 1.0222x over previous
"""CNF forward (vector field + exact Jacobian trace) on 8 TRN2 cores.

Math: reference computes, per sample x (row of state[:, 1:]):
    f(x)  = W3^T tanh(W2^T tanh(W1^T [x; t] + b1) + b2) + b3      (dx)
    trJ   = trace(df/dx)                                          (aug = -trJ)

Closed form of the trace (no JVP loop):
    h1 = tanh([x;t] @ W1 + b1),  h2 = tanh(h1 @ W2 + b2)
    s1 = 1 - h1^2,               s2 = 1 - h2^2
    trJ = s1^T F s2   with  F[h',h] = W2[h',h] * (W3 @ W1[:D])[h, h']

Sharding: data-parallel, 128 samples per core, weights replicated.

All matmuls run in bf16 (1 PE cycle/row vs fp32's 4), accumulating in
fp32 PSUM; host-side work is layout/dtype-only. The b1/t bias rides the
contraction as a K=65 ones-row, so layer 1 is four back-to-back matmuls
into one PSUM bank finished by a single 512-wide tanh. Weights stream as
five DMA blocks over three engine rings: the layer-1 block first, W2 in
four 128KB tiles whose completion fences fire as each tile lands (a tiny
identity shim keeps their descriptors from head-of-line blocking the
layer-1 block). PE order: z1 -> e2t (W2 still in flight) -> z2 -> t2 ->
h2 transposes -> dx; t2_k chases F_k so the trace matmul starts before
the last F tile exists. Engine split honoring "gpsimd can't read PSUM":
DVE does F = W2 .* E2T and the trace product+reduce, gpsimd does the
tanh' terms from SBUF, ACT does tanhs and all PSUM->SBUF copies.

NOTE: DVE tensor_tensor_reduce faults real HW (NRT unrecoverable;
CoreSim accepts it) — use the tensor_mul + tensor_reduce pair instead.
"""

import numpy as np
import ml_dtypes

import concourse.bacc as bacc
import concourse.bass as bass
import concourse.tile as tile
from concourse import mybir
from concourse.bass_utils import run_bass_kernel_spmd
from concourse.tile_rust import add_dep_helper

B, D, H = 1024, 64, 512
NCORES = 8
BC = B // NCORES  # 128 samples per core
KT = H // 128     # 4 feature tiles of 128
F32 = mybir.dt.float32
BF16 = mybir.dt.bfloat16
AF = mybir.ActivationFunctionType
ALU = mybir.AluOpType
ts = bass.ts
BF = ml_dtypes.bfloat16

_NC = {}

# blkA column offsets: xT | w1ext  (65 partitions, bf16).
# Row 64 is the bias row: 1s under xT, b1+t*W1[D] under w1ext.
XT_OFF = 0
W1X_OFF = BC            # 128
BLKA_COLS = BC + H      # 640


def _build(with_bias23: bool):
    """with_bias23: include rank-1 bias adds for b2/b3 (batch-major layers
    can't take a per-free-dim bias via ACT). setup_inputs() has zero
    biases so the fast path skips them; nonzero b2/b3 still works."""
    nc = bacc.Bacc()

    blkA = nc.declare_dram_parameter("blkA", [D + 1, BLKA_COLS], BF16,
                                     isOutput=False)
    w3Tp = nc.declare_dram_parameter("w3Tp", [D, H], BF16, isOutput=False)
    w2d = [nc.declare_dram_parameter(f"w2_{k}", [128, H], BF16,
                                     isOutput=False) for k in range(3)]
    # w2_3 | w3 tiles side by side
    w2e = nc.declare_dram_parameter("w2e", [128, H + KT * D], BF16,
                                    isOutput=False)
    identB = nc.declare_dram_parameter("identB", [128, 128], BF16,
                                       isOutput=False)
    if with_bias23:
        b2r = nc.declare_dram_parameter("b2r", [1, H], BF16, isOutput=False)
        b3r = nc.declare_dram_parameter("b3r", [1, D], BF16, isOutput=False)
    out = nc.declare_dram_parameter("out", [BC, D + 1], F32, isOutput=True)

    with tile.TileContext(nc) as tc:
        with (
            tc.tile_pool(name="const", bufs=1) as cp,
            tc.tile_pool(name="act", bufs=1) as ap,
            tc.tile_pool(name="ps", bufs=1, space="PSUM") as ps,
        ):
            # ---- loads ----
            # sync: blkA (fast DIRECT2D gen), w2_2, out store
            # scalar: w3T (needed by e2t), W2 halves, ident
            # gpsimd: w2_3|w3 via SWDGE (generation overlaps the blkA wait)
            blkA_sb = cp.tile([D + 1, BLKA_COLS], BF16, tag="blkA")
            w3T_sb = cp.tile([D, H], BF16, tag="w3T")
            ident = cp.tile([128, 128], BF16, tag="ident")
            w2_sb = [cp.tile([128, H], BF16, tag=f"w2s_{k}", name=f"w2s_{k}")
                     for k in range(3)]
            w2e_sb = cp.tile([128, H + KT * D], BF16, tag="w2e")
            nc.sync.dma_start(out=blkA_sb, in_=blkA[:, :])
            nc.scalar.dma_start(out=w3T_sb, in_=w3Tp[:, :])
            nc.scalar.dma_start(out=w2_sb[0], in_=w2d[0][:, :])
            nc.scalar.dma_start(out=w2_sb[1], in_=w2d[1][:, :])
            nc.sync.dma_start(out=w2_sb[2], in_=w2d[2][:, :])
            nc.gpsimd.dma_start(out=w2e_sb, in_=w2e[:, :])
            nc.scalar.dma_start(out=ident, in_=identB[:, :])
            w2_sb.append(w2e_sb[:, 0:H])
            w3_sb = [w2e_sb[:, H + k * D:H + (k + 1) * D] for k in range(KT)]
            if with_bias23:
                b2r_sb = cp.tile([1, H], BF16, tag="b2r")
                nc.sync.dma_start(out=b2r_sb, in_=b2r[:, :])
                b3r_sb = cp.tile([1, D], BF16, tag="b3r")
                nc.sync.dma_start(out=b3r_sb, in_=b3r[:, :])
                onesr = cp.tile([1, BC], BF16, tag="onesr")
                nc.vector.memset(onesr, 1.0)
            xTe = blkA_sb[:, XT_OFF:XT_OFF + BC]            # [65, BC]
            w1e = blkA_sb[:, W1X_OFF:W1X_OFF + H]           # [65, H]
            w1x = blkA_sb[0:D, W1X_OFF:W1X_OFF + H]         # [64, H]
            w3T = w3T_sb[:, :]                              # [64, H]

            # ---- layer 1 (feature-major): 4 matmuls, one tanh ----
            z1_ps = ps.tile([128, KT * BC], F32, tag="z1", bufs=1)
            z1_mm = []
            for j in range(KT):
                z1_mm.append(
                    nc.tensor.matmul(z1_ps[:, ts(j, BC)], w1e[:, ts(j, 128)],
                                     xTe, start=True, stop=True))
            h1 = ap.tile([128, KT * BC], BF16, tag="h1")
            s1 = ap.tile([128, KT * BC], BF16, tag="s1")
            nc.scalar.activation(h1, z1_ps, AF.Tanh)
            nc.gpsimd.tensor_mul(s1, h1, h1)
            nc.gpsimd.tensor_scalar(s1, s1, -1.0, 1.0, ALU.mult, ALU.add)

            # ---- trace weight matrix F (runs while W2 is in flight) ----
            # ACT drains each e2t PSUM slot to bf16 SBUF (faster than a DVE
            # read-modify chain would free it), then the DVE multiply runs
            # all-bf16 (2x mode) off the critical path.
            f_sb = []
            e2t_mm = []
            for m in range(KT):
                e2t_ps = ps.tile([128, H], F32, tag="e2t", bufs=2)
                mm = nc.tensor.matmul(e2t_ps, w1x[:, ts(m, 128)], w3T,
                                      start=True, stop=True)
                if m == 0:
                    add_dep_helper(mm.ins, z1_mm[KT - 1].ins, sync=False,
                                   reason="pe-order e2t after z1")
                e2t_mm.append(mm)
                e2c = ap.tile([128, H], BF16, tag=f"e2c_{m}", name=f"e2c_{m}")
                nc.scalar.copy(e2c, e2t_ps)
                fm = ap.tile([128, H], BF16, tag=f"f_{m}")
                nc.vector.tensor_mul(fm, w2_sb[m], e2c)
                f_sb.append(fm)

            # ---- layer 2 (batch-major): h2, s2 ----
            z2_ps = ps.tile([BC, H], F32, tag="z2", bufs=1)
            z2_mm = []
            for k in range(KT):
                z2_mm.append(
                    nc.tensor.matmul(z2_ps, h1[:, ts(k, BC)], w2_sb[k],
                                     start=(k == 0),
                                     stop=(k == KT - 1 and not with_bias23)))
            add_dep_helper(z2_mm[0].ins, e2t_mm[KT - 1].ins, sync=False,
                           reason="pe-order z2 after e2t")
            if with_bias23:
                nc.tensor.matmul(z2_ps, onesr, b2r_sb, start=False, stop=True)
            h2 = ap.tile([BC, H], BF16, tag="h2")
            s2 = ap.tile([BC, H], BF16, tag="s2")
            HH = H // 2
            # tanh in halves so the first transpose starts sooner
            nc.scalar.activation(h2[:, 0:HH], z2_ps[:, 0:HH], AF.Tanh)
            nc.scalar.activation(h2[:, HH:H], z2_ps[:, HH:H], AF.Tanh)
            # s2 on DVE (all-bf16, 2x-eligible), chained after each tanh
            # half so the trace product is never waiting on it
            nc.vector.tensor_mul(s2[:, 0:HH], h2[:, 0:HH], h2[:, 0:HH])
            nc.vector.tensor_scalar(s2[:, 0:HH], s2[:, 0:HH],
                                    -1.0, 1.0, ALU.mult, ALU.add)
            nc.vector.tensor_mul(s2[:, HH:H], h2[:, HH:H], h2[:, HH:H])
            nc.vector.tensor_scalar(s2[:, HH:H], s2[:, HH:H],
                                    -1.0, 1.0, ALU.mult, ALU.add)

            # ---- trJ = s1^T F s2 (batch-major) ----
            t2_ps = ps.tile([BC, H], F32, tag="t2", bufs=1)
            t2_mm = []
            for k in range(KT):
                t2_mm.append(
                    nc.tensor.matmul(t2_ps, s1[:, ts(k, BC)], f_sb[k],
                                     start=(k == 0), stop=(k == KT - 1)))
            add_dep_helper(t2_mm[0].ins, z2_mm[KT - 1].ins, sync=False,
                           reason="pe-order t2 after z2")
            final_sb = ap.tile([BC, D + 1], F32, tag="final")
            ttr_scr = ap.tile([BC, H], BF16, tag="ttr_scr")
            nc.vector.tensor_mul(ttr_scr, t2_ps, s2)
            nc.vector.tensor_reduce(out=final_sb[:, 0:1], in_=ttr_scr,
                                    op=ALU.add, axis=mybir.AxisListType.X,
                                    negate=True)

            # ---- layer 3 (batch-major): dx ----
            # double-buffered transpose PSUM; copies on ACT (the DVE owns
            # the trace product/reduce in this window)
            h2T_sb = []
            tr_mm = []
            for j in range(KT):
                hT_ps = ps.tile([128, BC], BF16, tag="tr", bufs=2)
                mm = nc.tensor.transpose(hT_ps, h2[:, ts(j, 128)], ident)
                if j == 0:
                    add_dep_helper(mm.ins, t2_mm[KT - 1].ins, sync=False,
                                   reason="pe-order transposes after t2")
                tr_mm.append(mm)
                hT = ap.tile([128, BC], BF16, tag=f"h2T_{j}", name=f"hT_{j}")
                nc.scalar.copy(hT, hT_ps)
                h2T_sb.append(hT)
            o_ps = ps.tile([BC, D], F32, tag="o", bufs=1)
            for k in range(KT):
                nc.tensor.matmul(o_ps, h2T_sb[k], w3_sb[k],
                                 start=(k == 0),
                                 stop=(k == KT - 1 and not with_bias23))
            if with_bias23:
                nc.tensor.matmul(o_ps, onesr, b3r_sb, start=False, stop=True)
            nc.scalar.copy(final_sb[:, 1:D + 1], o_ps)
            nc.sync.dma_start(out=out[:, :], in_=final_sb)

    nc.finalize()
    return nc


def _get_nc(with_bias23: bool):
    key = bool(with_bias23)
    if key not in _NC:
        _NC[key] = _build(key)
    return _NC[key]


def make_in_maps(inputs):
    f32 = lambda a: np.ascontiguousarray(np.asarray(a), dtype=np.float32)
    bfc = lambda a: np.ascontiguousarray(a.astype(BF))
    state = f32(inputs["state"])
    t = float(np.asarray(inputs["t"]).reshape(-1)[0])
    W1 = f32(inputs["W1"])
    b1 = f32(inputs["b1"]).reshape(H)
    W2 = f32(inputs["W2"])
    b2 = f32(inputs["b2"]).reshape(H)
    W3 = f32(inputs["W3"])
    b3 = f32(inputs["b3"]).reshape(D)

    with_bias23 = bool(np.any(b2) or np.any(b3))

    b1_eff = b1 + t * W1[D]                       # fold t-row into bias
    w1e = np.concatenate([W1[:D], b1_eff[None, :]], axis=0)   # [65, H]
    w2b = W2.astype(BF)
    w3blk = np.concatenate([W3[k * 128:(k + 1) * 128] for k in range(KT)],
                           axis=1).astype(BF)     # [128, KT*D]
    base = {
        "w2_0": np.ascontiguousarray(w2b[0:128]),
        "w2_1": np.ascontiguousarray(w2b[128:256]),
        "w2_2": np.ascontiguousarray(w2b[256:384]),
        "w2e": np.ascontiguousarray(
            np.concatenate([w2b[384:512], w3blk], axis=1)),
        "identB": np.eye(128, dtype=BF),
        "w3Tp": np.ascontiguousarray(W3.T).astype(BF),
    }
    if with_bias23:
        base["b2r"] = bfc(b2.reshape(1, H))
        base["b3r"] = bfc(b3.reshape(1, D))
    ones_row = np.ones((1, BC), np.float32)
    in_maps = []
    for c in range(NCORES):
        m = dict(base)
        xT = state[c * BC:(c + 1) * BC, 1:].T                 # [64, BC]
        xTe = np.concatenate([xT, ones_row], axis=0)          # [65, BC]
        m["blkA"] = np.ascontiguousarray(np.concatenate(
            [xTe, w1e], axis=1).astype(BF))
        in_maps.append(m)
    return with_bias23, in_maps


def kernel(**inputs) -> np.ndarray:
    with_bias23, in_maps = make_in_maps(inputs)
    res = run_bass_kernel_spmd(_get_nc(with_bias23), in_maps,
                               list(range(NCORES))).results
    return np.concatenate([res[c]["out"] for c in range(NCORES)], axis=0)
